# revision 1
# baseline (speedup 1.0000x reference)
"""ARIMA(0,1,1) innovations kernel for 8 TRN2 NeuronCores.

Math: the reference solves the min-norm least-squares problem A x = b where
A is the N x (N+1) bidiagonal MA(1) matrix (c on the diagonal, 1 on the
superdiagonal), b = diff(time_block) - arma_const, and returns x / std.

With s = -c, every solution satisfies x_{i+1} = s*x_i + b_i and the
min-norm one is x_i = xhat_i - rho*s^i with xhat the zero-init IIR scan of
b and rho = sum_j b_j s^{j+1} (exact to f32 for |c| < 1).

Layout: b is blocked [128, 32] (partition p holds elements 32p..32p+31).
The per-block initial states init[p] = x_{32p} are produced directly as
one PSUM column by two accumulating K=128 bf16 matmuls:

    init[p] = sum_q M2[q,p]*floc31[q] + sum_q B2[q,p]*g[q]
    M2[q,p] = istd*s^{32(p-1-q)}   (p>=q+1; else 0)  -- carry propagation
    B2[q,p] = s^{32(p+q)}                            -- rank-one rho term
    g[q]    = sum_k B1[q,k] * (-istd*s^{k+1})        -- fused mult+row-sum

where floc31[q] is the final value of the local forward scan of b-block q.
x_0 = init[0] falls out of the same matmuls.  The output is stored as one
[128,32] DMA for out[1:4097] (sync queue) plus a 16-descriptor block-head
scatter of out[32p], p<16 (scalar queue) that covers x_0 — a plain 4-byte
store measured ~1.1us of queue time, and the scatter issues right after
the PSUM copy, before the final scan retires.

The three scalar parameters are COMPILE-TIME IMMEDIATES (the NEFF is
rebuilt per parameter set inside kernel(); compile time is host-side and
free), so the single input DMA is time_block itself.  Everything scalar-
derived is built on device during the ~2.3us input-DMA dead window:
exponent matrices for M2/B2 via iota + affine_select on gpsimd, the Exp
activation table via a warmup activation, M2/B2 themselves via one Exp
activation each  exp(E*ln|s| + ln(istd))  (even exponents throughout, so
no sign handling; the signed s^{k+1} g-weights come from a dead-window
scan seeded with -istd), and the final-scan input b*istd on gpsimd.

Raw bass (Block + manual semaphores): DVE self-chains via `vs` (the DVE
pipe does not interlock same-engine RAW; neither does Pool), the PE drain
publishes `pp`, gpsimd `es`/`ws`, the scalar engine `aa`.  No
collectives: the problem is 16 KB in/out, so all 8 cores run the
identical program (data-parallel replication per the sharding hint) and
the host takes core 0's output.

Assumes 0 < |ma_coeff| < 1 (reference setup uses c = 0.5; at |c| -> 1 the
geometric-series identity for the projection coefficient degrades).
"""

import numpy as np

N = 4096
P = 128
Q = 32

_CACHE: dict = {}


def _ensure_paths():
    import sys
    for p in ("/opt/trn_rl_repo", "/root/.axon_site", "/root/.axon_site/_ro/trn_rl_repo",
              "/root/.axon_site/_ro/pypackages"):
        if p not in sys.path:
            sys.path.append(p)


def build_nc_raw(c: float, const: float, std: float):
    _ensure_paths()
    from contextlib import ExitStack
    import concourse.bass as bass
    import concourse.mybir as mybir

    f32 = mybir.dt.float32
    bf16 = mybir.dt.bfloat16
    OP = mybir.AluOpType
    EXP = mybir.ActivationFunctionType.Exp

    s = float(-c)
    istd = float(1.0 / std)
    ln_s = float(np.log(abs(s)))
    ln_istd = float(np.log(istd))

    nc = bass.Bass()

    tb_d = nc.dram_tensor("time_block", [N + 1], f32, kind="ExternalInput")
    out_d = nc.dram_tensor("out", [N + 1], f32, kind="ExternalOutput")

    ctx = ExitStack()
    t = lambda name, shape, dt=f32: ctx.enter_context(nc.sbuf_tensor(name, shape, dt))
    with ctx:
        TB33 = t("TB33", [P, Q + 1])     # TB33[p, j] = tb[32p + j]
        E1 = t("E1", [P, P])             # 32(p-1-q), +3e38 where p<=q
        E2 = t("E2", [P, P])             # 32(p+q)
        M2 = t("M2", [P, P], bf16)       # exp(E1*lnS + lnIstd)
        B2 = t("B2", [P, P], bf16)       # exp(E2*lnS)
        LNS = t("LNS", [P, 1])           # ln|s| (activation scale ptr)
        LNI = t("LNI", [P, 1])           # ln(istd) (activation bias ptr)
        Z1 = t("Z1", [1, 1])             # exp-table warmup scratch
        Zo = t("Zo", [1, 1])
        SC = t("SC", [P, 1])             # s (scan multiplier column)
        Z32 = t("Z32", [P, Q])           # zeros (g-weight generator src)
        W32 = t("W32", [P, Q])           # -istd*s^{k+1}
        B1 = t("B1", [P, Q])             # b = diff(tb) - const (unscaled)
        FLB = t("FLB", [P, Q], bf16)     # local forward scan, bf16
        WP = t("WP", [P, Q])             # B1 * W32 (accum feeds G)
        G = t("G", [P, 1], bf16)         # row sums of WP (fused accum)
        BS = t("BS", [P, Q])             # B1 * istd (gpsimd)
        FF33 = t("FF33", [P, Q + 1])     # col0 = x_{32p}; cols 1..32 = scan

        psC = ctx.enter_context(nc.psum_tensor("psC", [P, 1], f32))

        dS = ctx.enter_context(nc.semaphore("dS"))
        dA = ctx.enter_context(nc.semaphore("dA"))
        vs = ctx.enter_context(nc.semaphore("vs"))
        pp = ctx.enter_context(nc.semaphore("pp"))
        ws = ctx.enter_context(nc.semaphore("ws"))
        es = ctx.enter_context(nc.semaphore("es"))
        aa = ctx.enter_context(nc.semaphore("aa"))

        blk = ctx.enter_context(nc.Block())

        import bass_rust as _br
        tb_overlap = _br.AP(tb_d[0:1].tensor, 0, [[Q, P], [1, Q + 1]])

        @blk.sync
        def _(sync):
            sync.dma_start(out=TB33[:], in_=tb_overlap).then_inc(dS, 16)
            sync.dma_start(
                out=out_d[1:N + 1].rearrange("(p q) -> p q", p=P), in_=FF33[:, 1:Q + 1]
            )._wait_ge(vs, 9).then_inc(dS, 16)

        @blk.scalar
        def _(scalar):
            A = nc.scalar
            A.activation(Zo[:], Z1[:], EXP)._wait_ge(ws, 1)  # exp-table warmup
            scalar.wait_ge(es, 5)
            A.activation(M2[:], E1[:], EXP, bias=LNI[:, 0:1],
                         scale=LNS[:, 0:1]).then_inc(aa, 1)
            A.activation(B2[:], E2[:], EXP, bias=0.0,
                         scale=LNS[:, 0:1]).then_inc(aa, 1)
            # Block-head scatter of out[32p], p<16 (the sync-queue DMA covers
            # out[1:4097]; only x0 is unique here).  A 4-byte store costs
            # ~1.1us of queue time and a 16-row block costs ACT a ~390ns DGE
            # drain at the barrier; 16 scattered 4B descriptors issued early
            # (right after the PSUM copy, before the scan retires) keep the
            # slow ACT queue entirely off the critical path.
            with nc.allow_non_contiguous_dma("16 x 4B block-head scatter"):
                scalar.dma_start(
                    out=_br.AP(out_d[0:1].tensor, 0, [[Q, 16], [1, 1]]),
                    in_=FF33[0:16, 0:1]
                )._wait_ge(vs, 8).then_inc(dA, 16)

        @blk.gpsimd
        def _(gpsimd):
            G_ = nc.gpsimd
            G_.memset(Z1[:], 0.0).then_inc(ws, 1)
            G_.iota(E2[:], pattern=[[32, P]], base=0, channel_multiplier=32,
                    allow_small_or_imprecise_dtypes=True).then_inc(es, 1)
            G_.iota(E1[:], pattern=[[32, P]], base=-32, channel_multiplier=-32,
                    allow_small_or_imprecise_dtypes=True).then_inc(es, 1)
            G_.affine_select(E1[:], E1[:], pattern=[[1, P]],
                             compare_op=mybir.AluOpType.is_ge, fill=3e38,
                             base=-1, channel_multiplier=-1)._wait_ge(es, 2).then_inc(es, 1)
            G_.memset(LNS[:], ln_s).then_inc(es, 1)
            G_.memset(LNI[:], ln_istd).then_inc(es, 1)

        @blk.vector
        def _(vector):
            V = nc.vector
            # ---- dead-window constant builds (no input dependency) ----
            V.memset(SC[:], s).then_inc(vs, 1)                            # 1
            V.memset(Z32[:], 0.0).then_inc(vs, 1)                         # 2
            V.tensor_tensor_scan(
                W32[:], SC[:, 0:1].broadcast_to((P, Q)), Z32[:], -istd,
                OP.mult, OP.add
            )._wait_ge(vs, 2).then_inc(vs, 1)                             # 3
            # ---- input-dependent chain ----
            V.scalar_tensor_tensor(
                B1[:], TB33[:, 1:Q + 1], -const, TB33[:, 0:Q], OP.add, OP.subtract
            )._wait_ge(dS, 16).then_inc(vs, 1)                            # 4
            V.tensor_tensor_scan(
                FLB[:], SC[:, 0:1].broadcast_to((P, Q)), B1[:], 0.0, OP.mult, OP.add
            )._wait_ge(vs, 4).then_inc(vs, 1)                             # 5
            with nc.allow_low_precision("bf16 rho partials; 2e-2 rel-err budget"):
                V.scalar_tensor_tensor(
                    WP[:], B1[:], 1.0, W32[:], OP.mult, OP.mult, accum_out=G[:]
                )._wait_ge(vs, 5).then_inc(vs, 1)                         # 6
            V.tensor_scalar_mul(BS[:], B1[:], istd)._wait_ge(vs, 6).then_inc(vs, 1)  # 7
            V.tensor_copy(FF33[:, 0:1], psC[:, 0:1])._wait_ge(pp, 1).then_inc(vs, 1)  # 8
            V.tensor_tensor_scan(
                FF33[:, 1:Q + 1], SC[:, 0:1].broadcast_to((P, Q)), BS[:], psC[:, 0:1],
                OP.mult, OP.add
            )._wait_ge(vs, 7).then_inc(vs, 1)                             # 9

        @blk.tensor
        def _(tensor):
            T = nc.tensor
            tensor.wait_ge(aa, 1)
            T.matmul(psC[:], M2[:], FLB[:, Q - 1:Q], start=True,
                     stop=False)._wait_ge(vs, 5)
            tensor.wait_ge(aa, 2)
            T.matmul(psC[:], B2[:], G[:], start=False, stop=True)._wait_ge(vs, 6)
            T.drain().then_inc(pp, 1)

    return nc


def _get_nc(c: float, const: float, std: float):
    key = (c, const, std)
    if _CACHE.get("key") != key:
        _CACHE["nc"] = build_nc_raw(c, const, std)
        _CACHE["key"] = key
    return _CACHE["nc"]


def _in_map(inputs):
    return {
        "time_block": np.ascontiguousarray(
            np.asarray(inputs["time_block"], dtype=np.float32)
        ),
    }


def run(inputs, trace=False, tmpdir=None):
    """Run on all 8 cores (replicated); returns (output, BassKernelResults)."""
    _ensure_paths()
    from concourse.bass_utils import run_bass_kernel_spmd

    c = float(np.asarray(inputs["ma_coeff"]).reshape(-1)[0])
    const = float(np.asarray(inputs["arma_const"]).reshape(-1)[0])
    std = float(np.asarray(inputs["std_innovation"]).reshape(-1)[0])
    nc = _get_nc(c, const, std)
    m = _in_map(inputs)
    res = run_bass_kernel_spmd(nc, [m] * 8, list(range(8)), trace=trace, tmpdir=tmpdir)
    return res.results[0]["out"].reshape(N + 1).astype(np.float32), res


def kernel(**inputs) -> np.ndarray:
    out, _ = run(inputs)
    return out

